# revision 13
# baseline (speedup 1.0000x reference)
"""Fused multi-head attention layer on 8 trn2 NeuronCores.

Problem: B=2, N=2048, D=1024, H=16, HD=64 attention block:
  qkv proj -> per-head RMS norm (q,k) -> RoPE -> masked SDPA -> out proj
  -> row mask.

Sharding: sequence-parallel. Core c owns the 512 query tokens
[s*512,(s+1)*512) of batch b, with b=c//4, s=c%4. Each core computes
q/k/v projections + norm + rope for its own tokens, AllGathers the
post-rope kT and v slices among the 4 cores of its batch, runs full
16-head attention for its 512 queries over all 2048 keys, and the final
out-projection for its rows. The host concatenates the 8 [512,1024]
row-slices; no cross-core reduction is needed.

Numerics: matmuls run in float32r (fp32 data, fast PE streaming mode,
~1e-4 rel err). The head_dim axis is deinterleaved (evens then odds) via
a host-side permutation of the Wq/Wk columns and freqs so rotate-half
becomes a 32-partition block swap; RMS-norm weights, the 1/sqrt(HD)
score scale and rotate-half signs are folded into host-precomputed
cos/sin tables. The boolean key mask is applied as a -1e30 exp bias
(exp underflows to exactly 0); softmax denominators come from a ones
column appended to v, and the reciprocal is broadcast across partitions
with a tiny selector matmul.
"""

import numpy as np

import concourse.bass as bass
import concourse.mybir as mybir
from concourse.tile import TileContext
from concourse.bass_utils import run_bass_kernel_spmd

B, N, D = 2, 2048, 1024
H, HD = 16, 64
EPS = 1e-6
NT = N // 4            # 512 tokens owned per core
P = 128

dt = mybir.dt
MM_DT = dt.float32r    # matmul operand dtype
F32 = dt.float32


def _legalize_multiwaits(nc):
    """Split multi-wait instructions into single-wait NoOps + instruction.

    walrus in this toolchain rejects >1 sync wait on several instruction
    encodings ("Too many sync wait commands"). Same semantics: each extra
    wait goes on its own NoOp on the same engine right before the real
    instruction.
    """
    for fn in nc.m.functions:
        for bb in fn.blocks:
            out = []
            for ins in bb.instructions:
                si = ins.sync_info
                if si is not None and len(si.on_wait) > 1 and ins.engine is not None:
                    waits = list(si.on_wait)
                    for j, w in enumerate(waits[:-1]):
                        nop = mybir.InstNoOp(name=f"{ins.name}-lw{j}")
                        nop.engine = ins.engine
                        nop.sync_info = mybir.SyncInfo(on_wait=[w], on_update=[])
                        out.append(nop)
                    ins.sync_info = mybir.SyncInfo(
                        on_wait=[waits[-1]], on_update=list(si.on_update)
                    )
                out.append(ins)
            bb.instructions = out


def build_kernel():
    nc = bass.Bass(num_devices=8)

    # ---- dram parameters (per-core values supplied via in_maps) ----
    xT = nc.declare_dram_parameter("xT", [D, NT], MM_DT, isOutput=False)
    w3 = nc.declare_dram_parameter("w3", [D, 3 * D], MM_DT, isOutput=False)
    wo = nc.declare_dram_parameter("wo", [D, D], MM_DT, isOutput=False)
    cosq = nc.declare_dram_parameter("cosq", [NT, D], F32, isOutput=False)
    sinq = nc.declare_dram_parameter("sinq", [NT, D], F32, isOutput=False)
    cosk = nc.declare_dram_parameter("cosk", [NT, D], F32, isOutput=False)
    sink = nc.declare_dram_parameter("sink", [NT, D], F32, isOutput=False)
    mbias = nc.declare_dram_parameter("mbias", [N, 1], F32, isOutput=False)
    maskf = nc.declare_dram_parameter("maskf", [NT, 1], F32, isOutput=False)
    ident = nc.declare_dram_parameter("ident", [P, P], MM_DT, isOutput=False)
    onesd = nc.declare_dram_parameter("onesd", [P, 256], MM_DT, isOutput=False)
    out = nc.declare_dram_parameter("out", [NT, D], F32, isOutput=True)

    # ---- internal DRAM for the kT/v AllGather (4-core groups) ----
    # row 0: kT_own [D, NT] flat; row 1: v_own [NT, D] flat
    cc_in = nc.dram_tensor("cc_in", [2, D * NT], MM_DT)
    cc_out = nc.dram_tensor("cc_out", [8, D * NT], MM_DT)

    replica_groups = [[0, 1, 2, 3], [4, 5, 6, 7]]

    with TileContext(nc) as tc:
        import contextlib
        with contextlib.ExitStack() as ctx:
            # persistent pools (bottom of the SBUF stack)
            persist = ctx.enter_context(tc.tile_pool(name="persist", bufs=1))
            qkt = ctx.enter_context(tc.tile_pool(name="qkt", bufs=1))

            # constants
            ident_sb = persist.tile([P, P], MM_DT)
            nc.sync.dma_start(out=ident_sb, in_=ident[:, :])
            ones_sb = persist.tile([P, 256], MM_DT)
            nc.sync.dma_start(out=ones_sb, in_=onesd[:, :])
            ones64 = ones_sb[0:1, 0:64]
            # mask bias [128, 16]: column ckg = keys [ckg*128,(ckg+1)*128)
            mb_sb = persist.tile([P, 16], F32)
            nc.sync.dma_start(
                out=mb_sb, in_=mbias.ap().rearrange("(c p) o -> p (c o)", p=P))
            mf_sb = persist.tile([P, 4], F32)
            nc.sync.dma_start(
                out=mf_sb, in_=maskf.ap().rearrange("(c p) o -> p (c o)", p=P))
            eps_sb = persist.tile([P, 1], F32)
            nc.vector.memset(eps_sb, EPS)

            # q^T in [hd, token] layout, [128, dc=8, 512] (lives into attn)
            qT_sb = qkt.tile([P, 8, NT], MM_DT, name="qT")

            # Wo resident [128, dc=8, 1024] (loaded early, overlaps proj)
            wo_sb = persist.tile([P, 8, D], MM_DT)
            nc.sync.dma_start(
                out=wo_sb, in_=wo.ap().rearrange("(dc p) f -> p dc f", p=P))

            # attention-out per head-pair [128, 512] (out-proj lhsT), f32r
            outp_sb = [persist.tile([P, NT], MM_DT, name=f"outp{i}")
                       for i in range(8)]

            # ---------- phase 1: projections + norm + rope + transpose ----
            with tc.tile_pool(name="ph1", bufs=1) as ph1, \
                 tc.tile_pool(name="proj", bufs=3) as proj, \
                 tc.tile_pool(name="w3p", bufs=2) as w3p, \
                 tc.tile_pool(name="trig", bufs=2) as trig, \
                 tc.tile_pool(name="pps", bufs=3, space="PSUM") as pps, \
                 tc.tile_pool(name="tps", bufs=2, space="PSUM") as tps:

                # x^T resident [128, kc=8, 512] (phase-1 only)
                xT_sb = ph1.tile([P, 8, NT], MM_DT, name="xT")
                nc.sync.dma_start(
                    out=xT_sb, in_=xT.ap().rearrange("(kc p) t -> p kc t", p=P))
                # k^T(own) [hd, token] layout (phase-1 only; attn reads cc_out)
                kT_sb = ph1.tile([P, 8, NT], MM_DT, name="kT")

                def proj_block(fc, tc_i, w3t):
                    """psum [128 tok, 512 dims] = x[tok chunk] @ W3[:, fc]"""
                    ps = pps.tile([P, 512], F32, name="projps")
                    for kc in range(8):
                        nc.tensor.matmul(
                            ps,
                            xT_sb[:, kc, tc_i * P:(tc_i + 1) * P],
                            w3t[:, kc, :],
                            start=(kc == 0), stop=(kc == 7))
                    return ps

                def norm_rope_transpose(fc_rel, tc_i, ps, cos_d, sin_d, dst):
                    """RMS-norm + rope on psum [128 tok, 512 dims(8 heads)],
                    then transpose into dst [128, dc=8, 512] at the right spot.
                    fc_rel: 0/1 = which half of the 1024 q (or k) dims."""
                    sq = proj.tile([P, 512], F32, name="sq")
                    nc.scalar.activation(
                        sq, ps, mybir.ActivationFunctionType.Square)
                    ms = proj.tile([P, 8], F32, name="ms")
                    nc.vector.tensor_reduce(
                        ms, sq.rearrange("p (h d) -> p h d", h=8),
                        axis=mybir.AxisListType.X, op=mybir.AluOpType.add)
                    # sqrt(ms/64 + eps) then 1/x
                    rstd = proj.tile([P, 8], F32, name="rstd")
                    nc.scalar.activation(
                        rstd, ms, mybir.ActivationFunctionType.Sqrt,
                        bias=eps_sb, scale=1.0 / HD)
                    nc.vector.reciprocal(rstd, rstd)
                    qn = proj.tile([P, 512], F32, name="qn")
                    for h in range(8):
                        nc.vector.tensor_scalar_mul(
                            qn[:, h * HD:(h + 1) * HD],
                            ps[:, h * HD:(h + 1) * HD],
                            rstd[:, h:h + 1])
                    # rope: out = qn*cos + swap32(qn)*sin_s
                    cos_t = trig.tile([P, 512], F32, name="cos")
                    nc.sync.dma_start(
                        out=cos_t,
                        in_=cos_d[tc_i * P:(tc_i + 1) * P,
                                  fc_rel * 512:(fc_rel + 1) * 512])
                    sin_t = trig.tile([P, 512], F32, name="sin")
                    nc.sync.dma_start(
                        out=sin_t,
                        in_=sin_d[tc_i * P:(tc_i + 1) * P,
                                  fc_rel * 512:(fc_rel + 1) * 512])
                    shuf = proj.tile([P, 8, 2, 32], F32, name="shuf")
                    qn3 = qn.rearrange("p (h s d) -> p h s d", h=8, s=2)
                    nc.vector.tensor_copy(shuf[:, :, 0, :], qn3[:, :, 1, :])
                    nc.vector.tensor_copy(shuf[:, :, 1, :], qn3[:, :, 0, :])
                    m1 = proj.tile([P, 512], F32, name="m1")
                    nc.vector.tensor_tensor(
                        out=m1, in0=qn, in1=cos_t, op=mybir.AluOpType.mult)
                    m2 = proj.tile([P, 512], F32, name="m2")
                    nc.vector.tensor_tensor(
                        out=m2,
                        in0=shuf.rearrange("p h s d -> p (h s d)"),
                        in1=sin_t, op=mybir.AluOpType.mult)
                    rr = proj.tile([P, 512], MM_DT, name="rr")
                    nc.vector.tensor_add(rr, m1, m2)
                    # transpose 4 [128,128] blocks into dst[:, dc, tok]
                    for j in range(4):
                        tp = tps.tile([P, P], MM_DT, name="tp")
                        nc.tensor.transpose(
                            tp, rr[:, j * P:(j + 1) * P], ident_sb)
                        dc = fc_rel * 4 + j
                        nc.vector.tensor_copy(
                            dst[:, dc, tc_i * P:(tc_i + 1) * P], tp)

                # --- k (W3 cols 1024:2048) ---
                for fc in (2, 3):
                    w3t = w3p.tile([P, 8, 512], MM_DT, name="w3t")
                    nc.sync.dma_start(
                        out=w3t,
                        in_=w3.ap()[:, fc * 512:(fc + 1) * 512]
                        .rearrange("(kc p) f -> p kc f", p=P))
                    for tc_i in range(4):
                        ps = proj_block(fc, tc_i, w3t)
                        norm_rope_transpose(fc - 2, tc_i, ps, cosk, sink, kT_sb)
                # ship kT to cc_in row 0
                nc.sync.dma_start(
                    out=cc_in.ap()[0].rearrange("(dc p t) -> p dc t", p=P, t=NT),
                    in_=kT_sb)

                # --- v (W3 cols 2048:3072) ---
                for fc in (4, 5):
                    w3t = w3p.tile([P, 8, 512], MM_DT, name="w3t")
                    nc.sync.dma_start(
                        out=w3t,
                        in_=w3.ap()[:, fc * 512:(fc + 1) * 512]
                        .rearrange("(kc p) f -> p kc f", p=P))
                    for tc_i in range(4):
                        ps = proj_block(fc, tc_i, w3t)
                        vv = proj.tile([P, 512], MM_DT, name="vv")
                        nc.vector.tensor_copy(vv, ps)
                        # v flat [NT,1024]: (tok=tc*128+p, dim=(fc-4)*512+f)
                        nc.sync.dma_start(
                            out=cc_in.ap()[1]
                            .rearrange("(t4 p f2 f) -> t4 p f2 f",
                                       t4=4, p=P, f2=2)[tc_i, :, fc - 4, :],
                            in_=vv)

                # --- AllGather kT+v among the 4 cores of this batch ---
                nc.gpsimd.collective_compute(
                    "AllGather", mybir.AluOpType.bypass,
                    replica_groups=replica_groups,
                    ins=[cc_in[:, :]], outs=[cc_out[:, :]],
                )

                # --- q (W3 cols 0:1024), overlaps the AllGather ---
                for fc in (0, 1):
                    w3t = w3p.tile([P, 8, 512], MM_DT, name="w3t")
                    nc.sync.dma_start(
                        out=w3t,
                        in_=w3.ap()[:, fc * 512:(fc + 1) * 512]
                        .rearrange("(kc p) f -> p kc f", p=P))
                    for tc_i in range(4):
                        ps = proj_block(fc, tc_i, w3t)
                        norm_rope_transpose(fc, tc_i, ps, cosq, sinq, qT_sb)

            # ---------- phase 2: attention ----------
            with tc.tile_pool(name="kv", bufs=1) as kv, \
                 tc.tile_pool(name="att", bufs=2) as att, \
                 tc.tile_pool(name="rcp", bufs=1) as rcp, \
                 tc.tile_pool(name="scps", bufs=3, space="PSUM") as scps, \
                 tc.tile_pool(name="ops", bufs=2, space="PSUM") as ops, \
                 tc.tile_pool(name="rps", bufs=1, space="PSUM") as rps:

                # gathered kT [128, dc=8, 2048] and v(+ones) [128, 16, 16, 65]
                kt_all = kv.tile([P, 8, N], MM_DT, name="kt_all")
                v_aug = kv.tile([P, 16, H, 65], MM_DT, name="v_aug")
                nc.sync.dma_start(
                    out=v_aug[:, :, :, 64:65],
                    in_=onesd.ap().rearrange("p (c h o) -> p c h o", c=16, h=H))
                for s in range(4):
                    nc.sync.dma_start(
                        out=kt_all[:, :, s * NT:(s + 1) * NT],
                        in_=cc_out.ap()[2 * s]
                        .rearrange("(dc p t) -> p dc t", p=P, t=NT))
                    for ck in range(4):
                        nc.sync.dma_start(
                            out=v_aug[:, s * 4 + ck, :, 0:64],
                            in_=cc_out.ap()[2 * s + 1]
                            .rearrange("(c p h d) -> c p h d", c=4, p=P, h=H)[ck])

                for hp in range(8):          # head pairs
                    op_pair = [ops.tile([65, NT], F32, name="op") for _ in range(2)]
                    for s in range(4):       # source core (key slice)
                        for hh in range(2):  # head within pair
                            h = hp * 2 + hh
                            hb = (h % 2) * 64
                            for ck in range(4):
                                ckg = s * 4 + ck
                                sc = scps.tile([P, NT], F32, name="sc")
                                nc.tensor.matmul(
                                    sc,
                                    kt_all[hb:hb + 64, h // 2,
                                           ckg * P:(ckg + 1) * P],
                                    qT_sb[hb:hb + 64, h // 2, :],
                                    start=True, stop=True)
                                et = att.tile([P, NT], MM_DT, name="et")
                                nc.scalar.activation(
                                    et, sc, mybir.ActivationFunctionType.Exp,
                                    bias=mb_sb[:, ckg:ckg + 1], scale=1.0)
                                nc.tensor.matmul(
                                    op_pair[hh],
                                    v_aug[:, ckg, h, :],
                                    et,
                                    start=(ckg == 0), stop=(ckg == 15))
                    # normalize: recip of sums row, K=1 ones-matmul broadcast
                    with nc.allow_low_precision(reason="f32r == f32 bits"):
                        for hh in range(2):
                            rch = rcp.tile([1, NT], MM_DT, name=f"rc{hh}")
                            nc.vector.reciprocal(rch, op_pair[hh][64:65, :])
                            rcb = rps.tile([64, NT], F32, name=f"rcb{hh}")
                            nc.tensor.matmul(rcb, ones64, rch,
                                             start=True, stop=True)
                            rcs = rcp.tile([64, NT], F32, name=f"rcs{hh}")
                            nc.vector.tensor_copy(rcs, rcb)
                            nc.vector.tensor_tensor(
                                out=outp_sb[hp][hh * 64:(hh + 1) * 64, :],
                                in0=op_pair[hh][0:64, :],
                                in1=rcs,
                                op=mybir.AluOpType.mult)

            # ---------- phase 3: out projection + row mask ----------
            with tc.tile_pool(name="outp", bufs=3) as outp, \
                 tc.tile_pool(name="oppr", bufs=3, space="PSUM") as oppr:
                for tc_i in range(4):
                    for Fc in range(2):
                        ps = oppr.tile([P, 512], F32, name="ops2")
                        for dc in range(8):
                            nc.tensor.matmul(
                                ps,
                                outp_sb[dc][:, tc_i * P:(tc_i + 1) * P],
                                wo_sb[:, dc, Fc * 512:(Fc + 1) * 512],
                                start=(dc == 0), stop=(dc == 7))
                        ot = outp.tile([P, 512], F32, name="ot")
                        nc.vector.tensor_scalar_mul(
                            ot, ps, mf_sb[:, tc_i:tc_i + 1])
                        nc.sync.dma_start(
                            out=out[tc_i * P:(tc_i + 1) * P,
                                    Fc * 512:(Fc + 1) * 512],
                            in_=ot)

    _legalize_multiwaits(nc)
    return nc


def _prep_host(x, mask, freqs, Wq, Wk, Wv, Wo, q_norm_w, k_norm_w):
    """Host-side shard + fold. Returns per-core input maps."""
    perm = np.concatenate([np.arange(0, HD, 2), np.arange(1, HD, 2)])  # evens, odds
    swap = np.concatenate([np.arange(32, 64), np.arange(0, 32)])

    def permute_cols(W):  # [D, D] -> per-head column perm
        W4 = W.reshape(D, H, HD)
        return W4[:, :, perm].reshape(D, D)

    W3 = np.concatenate(
        [permute_cols(Wq), permute_cols(Wk), Wv], axis=1)  # [D, 3D]
    W3 = np.ascontiguousarray(W3, dtype=np.float32)
    Wo = np.ascontiguousarray(Wo, dtype=np.float32)

    fp = freqs[:, perm].astype(np.float32)          # [N, 64]
    cos = np.cos(fp)
    sin = np.sin(fp)
    sin_s = sin.copy()
    sin_s[:, :32] = -sin_s[:, :32]                  # rotate-half signs
    wq_p = q_norm_w[perm].astype(np.float32)
    wk_p = k_norm_w[perm].astype(np.float32)
    sc = 1.0 / np.sqrt(HD)
    cos_q = cos * wq_p * sc
    sin_q = sin_s * wq_p[swap] * sc
    cos_k = cos * wk_p
    sin_k = sin_s * wk_p[swap]
    # tile across the 16 heads -> [N, 1024]
    t16 = lambda a: np.tile(a, (1, H)).astype(np.float32)
    cos_q, sin_q, cos_k, sin_k = map(t16, (cos_q, sin_q, cos_k, sin_k))

    ident = np.eye(P, dtype=np.float32)

    in_maps = []
    for c in range(8):
        b, s = c // 4, c % 4
        rows = slice(s * NT, (s + 1) * NT)
        xT_c = np.ascontiguousarray(x[b, rows, :].T, dtype=np.float32)
        mbias = np.where(mask[b], 0.0, -1e30).astype(np.float32).reshape(N, 1)
        mf = mask[b, rows].astype(np.float32).reshape(NT, 1)
        in_maps.append(dict(
            xT=xT_c, w3=W3, wo=Wo,
            cosq=np.ascontiguousarray(cos_q[rows]),
            sinq=np.ascontiguousarray(sin_q[rows]),
            cosk=np.ascontiguousarray(cos_k[rows]),
            sink=np.ascontiguousarray(sin_k[rows]),
            mbias=mbias, maskf=mf, ident=ident,
            onesd=np.ones((P, 256), dtype=np.float32),
        ))
    return in_maps


_CACHED = {}


def kernel(x, mask, freqs, Wq, bq, Wk, bk, Wv, bv, Wo, bo,
           q_norm_w, k_norm_w, **_unused):
    x = np.asarray(x, dtype=np.float32)
    mask = np.asarray(mask)
    for name, b_ in (("bq", bq), ("bk", bk), ("bv", bv), ("bo", bo)):
        if np.any(np.asarray(b_) != 0):
            raise NotImplementedError(f"nonzero bias {name} not supported")

    in_maps = _prep_host(np.asarray(x), np.asarray(mask),
                         np.asarray(freqs, dtype=np.float32),
                         np.asarray(Wq, dtype=np.float32),
                         np.asarray(Wk, dtype=np.float32),
                         np.asarray(Wv, dtype=np.float32),
                         np.asarray(Wo, dtype=np.float32),
                         np.asarray(q_norm_w, dtype=np.float32),
                         np.asarray(k_norm_w, dtype=np.float32))

    if "nc" not in _CACHED:
        _CACHED["nc"] = build_kernel()
    res = run_bass_kernel_spmd(_CACHED["nc"], in_maps, core_ids=list(range(8)))

    out = np.empty((B, N, D), dtype=np.float32)
    for c in range(8):
        b, s = c // 4, c % 4
        out[b, s * NT:(s + 1) * NT, :] = res.results[c]["out"]
    return out


if __name__ == "__main__":
    # quick self-run with random inputs
    rng = np.random.default_rng(0)
    ins = dict(
        x=rng.standard_normal((B, N, D), dtype=np.float32),
        mask=rng.random((B, N)) < 0.5,
        freqs=rng.standard_normal((N, HD), dtype=np.float32),
        Wq=rng.standard_normal((D, D), dtype=np.float32) * 0.02,
        bq=np.zeros(D, np.float32),
        Wk=rng.standard_normal((D, D), dtype=np.float32) * 0.02,
        bk=np.zeros(D, np.float32),
        Wv=rng.standard_normal((D, D), dtype=np.float32) * 0.02,
        bv=np.zeros(D, np.float32),
        Wo=rng.standard_normal((D, D), dtype=np.float32) * 0.02,
        bo=np.zeros(D, np.float32),
        q_norm_w=np.ones(HD, np.float32),
        k_norm_w=np.ones(HD, np.float32),
    )
    o = kernel(**ins)
    print("out", o.shape, o.dtype, float(np.abs(o).max()))


# revision 27
# speedup vs baseline: 342.3444x; 342.3444x over previous
"""Fused multi-head attention layer on 8 trn2 NeuronCores.

Problem: B=2, N=2048, D=1024, H=16, HD=64 attention block:
  qkv proj -> per-head RMS norm (q,k) -> RoPE -> masked SDPA -> out proj
  -> row mask.

Sharding: sequence-parallel. Core c owns the 512 query tokens
[s*512,(s+1)*512) of batch b, with b=c//4, s=c%4. Each core computes
q/k/v projections + norm + rope for its own tokens, AllGathers the
post-rope kT and v slices among the 4 cores of its batch (split into two
head-group pieces so attention can start on heads 0-7 while heads 8-15
are still in flight), runs full attention for its 512 queries over all
2048 keys, and the final out-projection for its rows. The host
concatenates the 8 [512,1024] row-slices; no cross-core reduction.

Numerics: matmuls run in float32r (fp32 data, fast PE streaming mode,
~1e-4 rel err). The head_dim axis is deinterleaved (evens then odds) via
a host-side permutation of the Wq/Wk columns and freqs so rotate-half
becomes a 32-lane block swap; RMS-norm weights, the 1/sqrt(HD) score
scale and rotate-half signs are folded into host-precomputed cos/sin
tables (RMS rstd itself is applied after rope - rope is linear and the
scale is constant within a head). The boolean key mask is applied as a
-1e30 exp bias (exp underflows to exactly 0); softmax denominators come
from a ones column appended to v, and the reciprocal is broadcast
across partitions with a K=1 ones-matmul.
"""

import numpy as np

import concourse.bass as bass
import concourse.mybir as mybir
from concourse.tile import TileContext
from concourse.bass_utils import run_bass_kernel_spmd

B, N, D = 2, 2048, 1024
H, HD = 16, 64
EPS = 1e-6
NT = N // 4            # 512 tokens owned per core
P = 128
HG = D * NT // 2       # elements per half (kT or v) piece: [512,512]x2

dt = mybir.dt
MM_DT = dt.float32r    # matmul operand dtype (projections/transposes)
AT_DT = dt.bfloat16    # attention-path operand dtype (kT/v/exp/qT/Wo/outT)
F32 = dt.float32


def _legalize_multiwaits(nc):
    """Split multi-wait instructions into single-wait NoOps + instruction.

    walrus in this toolchain rejects >1 sync wait on several instruction
    encodings ("Too many sync wait commands"). Same semantics: each extra
    wait goes on its own NoOp on the same engine right before the real
    instruction.
    """
    for fn in nc.m.functions:
        for bb in fn.blocks:
            out = []
            for ins in bb.instructions:
                si = ins.sync_info
                if si is not None and len(si.on_wait) > 1 and ins.engine is not None:
                    waits = list(si.on_wait)
                    for j, w in enumerate(waits[:-1]):
                        nop = mybir.InstNoOp(name=f"{ins.name}-lw{j}")
                        nop.engine = ins.engine
                        nop.sync_info = mybir.SyncInfo(on_wait=[w], on_update=[])
                        out.append(nop)
                    ins.sync_info = mybir.SyncInfo(
                        on_wait=[waits[-1]], on_update=list(si.on_update)
                    )
                out.append(ins)
            bb.instructions = out


def build_kernel(fake_cc=False):
    nc = bass.Bass(num_devices=8)

    # ---- dram parameters (per-core values supplied via in_maps) ----
    xT = nc.declare_dram_parameter("xT", [D, NT], MM_DT, isOutput=False)
    w3 = nc.declare_dram_parameter("w3", [D, 3 * D], MM_DT, isOutput=False)
    wo = nc.declare_dram_parameter("wo", [D, D], MM_DT, isOutput=False)
    cosq = nc.declare_dram_parameter("cosq", [NT, HD], F32, isOutput=False)
    sinq = nc.declare_dram_parameter("sinq", [NT, HD], F32, isOutput=False)
    cosk = nc.declare_dram_parameter("cosk", [NT, HD], F32, isOutput=False)
    sink = nc.declare_dram_parameter("sink", [NT, HD], F32, isOutput=False)
    mbias = nc.declare_dram_parameter("mbias", [N, 1], F32, isOutput=False)
    maskf = nc.declare_dram_parameter("maskf", [NT, 1], F32, isOutput=False)
    ident = nc.declare_dram_parameter("ident", [P, P], MM_DT, isOutput=False)
    onesd = nc.declare_dram_parameter("onesd", [P, 256], MM_DT, isOutput=False)
    out = nc.declare_dram_parameter("out", [NT, D], F32, isOutput=True)

    # ---- internal DRAM for the split kT/v AllGathers (4-core groups) ----
    # piece g (g=0: heads 0-7, g=1: heads 8-15):
    #   row 0: kT rows [g*512,(g+1)*512) flat [512, NT]
    #   row 1: v cols  [g*512,(g+1)*512) flat [NT, 512]
    cc_in = [nc.dram_tensor(f"cc_in{g}", [2, HG], AT_DT) for g in range(2)]
    cc_out = [nc.dram_tensor(f"cc_out{g}", [8, HG], AT_DT) for g in range(2)]

    replica_groups = [[0, 1, 2, 3], [4, 5, 6, 7]]

    with TileContext(nc) as tc:
        import contextlib
        with contextlib.ExitStack() as ctx:
            # persistent pools (bottom of the SBUF stack)
            persist = ctx.enter_context(tc.tile_pool(name="persist", bufs=1))
            qkt = ctx.enter_context(tc.tile_pool(name="qkt", bufs=1))

            # critical-path first loads (xT + W3 k-half-0 chunk) get top
            # scheduler priority; constants follow
            early = {}

            # constants
            ident_sb = persist.tile([P, P], MM_DT)
            nc.sync.dma_start(out=ident_sb, in_=ident[:, :])
            ones_sb = persist.tile([1, 64], MM_DT)
            nc.sync.dma_start(out=ones_sb, in_=onesd[0:1, 0:64])
            ones64 = ones_sb[0:1, 0:64]
            # mask bias [128, 16]: column ckg = keys [ckg*128,(ckg+1)*128)
            mb_sb = persist.tile([P, 16], F32)
            nc.sync.dma_start(
                out=mb_sb, in_=mbias.ap().rearrange("(c p) o -> p (c o)", p=P))
            mf_sb = persist.tile([P, 4], F32)
            nc.sync.dma_start(
                out=mf_sb, in_=maskf.ap().rearrange("(c p) o -> p (c o)", p=P))
            eps_sb = persist.tile([P, 1], F32)
            nc.vector.memset(eps_sb, EPS)

            # q^T in [hd, token] layout, [128, dc=8, 512] (lives into attn)
            qT_sb = qkt.tile([P, 8, NT], AT_DT, name="qT")

            # Wo resident [128, dc=8, 1024] bf16; cast-DMA emitted late
            wo_sb = persist.tile([P, 8, D], AT_DT)

            # attention-out per head-pair [128, 512] (out-proj lhsT), bf16
            outp_sb = [persist.tile([P, NT], AT_DT, name=f"outp{i}")
                       for i in range(8)]

            # ---------- phases 1+2 share one scope so projections,
            # collectives and attention fully overlap ----------
            with tc.tile_pool(name="ph1", bufs=1) as ph1, \
                 tc.tile_pool(name="proj", bufs=3) as proj, \
                 tc.tile_pool(name="w3p", bufs=2) as w3p, \
                 tc.tile_pool(name="kv", bufs=1) as kv, \
                 tc.tile_pool(name="att", bufs=3) as att, \
                 tc.tile_pool(name="rcp", bufs=1) as rcp, \
                 tc.tile_pool(name="pps", bufs=2, space="PSUM") as pps, \
                 tc.tile_pool(name="tps", bufs=1, space="PSUM") as tps, \
                 tc.tile_pool(name="scps", bufs=2, space="PSUM") as scps, \
                 tc.tile_pool(name="ops", bufs=2, space="PSUM") as ops, \
                 tc.tile_pool(name="rps", bufs=1, space="PSUM") as rps:

                # x^T resident [128, kc=8, 512] (phase-1 only)
                xT_sb = ph1.tile([P, 8, NT], MM_DT, name="xT")
                # k^T(own) per head-group [128, dc=4, 512] (phase-1 only)
                kT_g = [ph1.tile([P, 4, NT], AT_DT, name=f"kT{g}")
                        for g in range(2)]
                # first W3 chunk early (before the trig tables) so its
                # DMA outranks non-critical loads
                _w3e = ph1.tile([P, 8, 512], MM_DT, name="w3e")
                for half in range(2):
                    nc.sync.dma_start(
                        out=_w3e[:, 4 * half:4 * half + 4, :],
                        in_=w3.ap()[half * 512:(half + 1) * 512,
                                    2 * 512:3 * 512]
                        .rearrange("(kc p) f -> p kc f", p=P))
                early["w3k0"] = _w3e
                # per-token-chunk trig tables [128, 64] (head-independent)
                trig_t = {}
                for kind, src in (("cq", cosq), ("sq", sinq),
                                  ("ck", cosk), ("sk", sink)):
                    for tci in range(4):
                        t = ph1.tile([P, HD], F32, name=f"tr_{kind}{tci}")
                        nc.sync.dma_start(
                            out=t, in_=src[tci * P:(tci + 1) * P, :])
                        trig_t[(kind, tci)] = t

                for kc in range(8):
                    nc.sync.dma_start(
                        out=xT_sb[:, kc, :],
                        in_=xT.ap()[kc * P:(kc + 1) * P, :])

                def load_w3(fc):
                    w3t = w3p.tile([P, 8, 512], MM_DT, name="w3t")
                    nc.sync.dma_start(
                        out=w3t,
                        in_=w3.ap()[:, fc * 512:(fc + 1) * 512]
                        .rearrange("(kc p) f -> p kc f", p=P))
                    return w3t

                def proj_block(tc_i, w3t):
                    """psum [128 tok, 512 dims] = x[tok chunk] @ W3[:, fc]"""
                    ps = pps.tile([P, 512], F32, name="projps")
                    for kc in range(8):
                        nc.tensor.matmul(
                            ps,
                            xT_sb[:, kc, tc_i * P:(tc_i + 1) * P],
                            w3t[:, kc, :],
                            start=(kc == 0), stop=(kc == 7))
                    return ps

                def norm_rope_transpose(g, tc_i, ps, ck_kind, dst, dst_dc0):
                    """RMS-norm + rope on psum [128 tok, 512 dims(8 heads)],
                    then transpose into dst[:, dst_dc0+j, tok]. g: 0/1 =
                    head-group = which half of the 1024 q/k dims."""
                    sq = proj.tile([P, 512], F32, name="sq")
                    nc.scalar.activation(
                        sq, ps, mybir.ActivationFunctionType.Square)
                    ms = proj.tile([P, 8], F32, name="ms")
                    nc.vector.tensor_reduce(
                        ms, sq.rearrange("p (h d) -> p h d", h=8),
                        axis=mybir.AxisListType.X, op=mybir.AluOpType.add)
                    rstd = proj.tile([P, 8], F32, name="rstd")
                    nc.scalar.activation(
                        rstd, ms, mybir.ActivationFunctionType.Sqrt,
                        bias=eps_sb, scale=1.0 / HD)
                    nc.vector.reciprocal(rstd, rstd)
                    # rope on the raw projection; rstd applied after (the
                    # rotation never mixes heads, so the per-(tok,head) scale
                    # commutes with it)
                    def rep8(t):
                        return bass.AP(tensor=t.tensor, offset=t.offset,
                                       ap=[list(t.ap[0]), [0, 8],
                                           list(t.ap[1])])
                    cos_t = trig_t[("c" + ck_kind, tc_i)]
                    sin_t = trig_t[("s" + ck_kind, tc_i)]
                    ps3 = ps.rearrange("p (h s d) -> p h s d", h=8, s=2)
                    m1 = proj.tile([P, 8, HD], F32, name="m1")
                    nc.vector.tensor_tensor(
                        out=m1, in0=ps.rearrange("p (h d) -> p h d", h=8),
                        in1=rep8(cos_t), op=mybir.AluOpType.mult)
                    # m2 = swap32(ps) * sin_s, built directly with strided APs
                    m2 = proj.tile([P, 8, 2, 32], F32, name="m2")
                    def rep8h(t, lo, hi):
                        return bass.AP(tensor=t.tensor, offset=t.offset + lo,
                                       ap=[list(t.ap[0]), [0, 8],
                                           [1, hi - lo]])
                    nc.vector.tensor_tensor(
                        out=m2[:, :, 0, :], in0=ps3[:, :, 1, :],
                        in1=rep8h(sin_t, 0, 32), op=mybir.AluOpType.mult)
                    nc.vector.tensor_tensor(
                        out=m2[:, :, 1, :], in0=ps3[:, :, 0, :],
                        in1=rep8h(sin_t, 32, 64), op=mybir.AluOpType.mult)
                    m2 = m2.rearrange("p h s d -> p h (s d)")
                    nc.vector.tensor_add(m1, m1, m2)
                    raw = m1.rearrange("p h d -> p (h d)")
                    # apply rstd per head (per-partition scalar broadcast)
                    rr = proj.tile([P, 512], MM_DT, name="rr")
                    for h in range(8):
                        nc.vector.tensor_scalar_mul(
                            rr[:, h * HD:(h + 1) * HD],
                            raw[:, h * HD:(h + 1) * HD],
                            rstd[:, h:h + 1])
                    # transpose 4 [128,128] blocks into dst[:, dc, tok]
                    for j in range(4):
                        tp = tps.tile([P, P], MM_DT, name="tp")
                        nc.tensor.transpose(
                            tp, rr[:, j * P:(j + 1) * P], ident_sb)
                        nc.vector.tensor_copy(
                            dst[:, dst_dc0 + j, tc_i * P:(tc_i + 1) * P], tp)

                def k_half(g, w3t=None):  # fc = 2 + g
                    if w3t is None:
                        w3t = load_w3(2 + g)
                    for tc_i in range(4):
                        ps = proj_block(tc_i, w3t)
                        norm_rope_transpose(g, tc_i, ps, "k", kT_g[g], 0)
                    nc.sync.dma_start(
                        out=cc_in[g].ap()[0]
                        .rearrange("(dc p t) -> p dc t", p=P, t=NT),
                        in_=kT_g[g])

                def v_half(g):  # fc = 4 + g
                    w3t = load_w3(4 + g)
                    for tc_i in range(4):
                        ps = proj_block(tc_i, w3t)
                        vv = proj.tile([P, 512], AT_DT, name="vv")
                        nc.vector.tensor_copy(vv, ps)
                        # v half flat [NT, 512]: (tok=tc*128+p, dim=f)
                        nc.sync.dma_start(
                            out=cc_in[g].ap()[1]
                            .rearrange("(t4 p f) -> t4 p f",
                                       t4=4, p=P)[tc_i],
                            in_=vv)

                def q_half(g):  # fc = g
                    w3t = load_w3(g)
                    for tc_i in range(4):
                        ps = proj_block(tc_i, w3t)
                        norm_rope_transpose(g, tc_i, ps, "q", qT_sb, 4 * g)

                def gather(g):
                    if fake_cc:
                        # timing-sim stand-in: same DRAM traffic, no collective
                        for s in range(4):
                            nc.sync.dma_start(
                                out=cc_out[g][2 * s:2 * s + 2, :],
                                in_=cc_in[g][:, :])
                        return
                    nc.gpsimd.collective_compute(
                        "AllGather", mybir.AluOpType.bypass,
                        replica_groups=replica_groups,
                        ins=[cc_in[g][:, :]], outs=[cc_out[g][:, :]],
                    )

                k_half(0, w3t=early["w3k0"])
                v_half(0)
                gather(0)
                q_half(0)
                k_half(1)
                v_half(1)
                gather(1)
                # Wo load rides the DMA lull during attention (casts)
                nc.gpsimd.dma_start(
                    out=wo_sb,
                    in_=wo.ap().rearrange("(dc p) f -> p dc f", p=P))
                q_half(1)

                # ---------- attention (per head-group) ----------
                for g in range(2):
                    # gathered kT [128, dc=4, 2048]; v(+ones) [128, 16, 8, 65]
                    kt_all = kv.tile([P, 4, N], AT_DT, name=f"kt{g}")
                    v_aug = kv.tile([P, 16, 8, 65], AT_DT, name=f"va{g}")
                    nc.vector.memset(v_aug[:, :, :, 64:65], 1.0)
                    for s in range(4):
                        nc.sync.dma_start(
                            out=kt_all[:, :, s * NT:(s + 1) * NT],
                            in_=cc_out[g].ap()[2 * s]
                            .rearrange("(dc p t) -> p dc t", p=P, t=NT))
                        for ck in range(4):
                            nc.sync.dma_start(
                                out=v_aug[:, s * 4 + ck, :, 0:64],
                                in_=cc_out[g].ap()[2 * s + 1]
                                .rearrange("(c p h d) -> c p h d",
                                           c=4, p=P, h=8)[ck])

                    for hpl in range(4):     # head pair within group
                        hp = g * 4 + hpl
                        op_pair = [ops.tile([65, NT], F32, name="op")
                                   for _ in range(2)]
                        for s in range(4):
                            for hh in range(2):
                                h = hpl * 2 + hh       # head within group
                                hb = (h % 2) * 64
                                for ck in range(4):
                                    ckg = s * 4 + ck
                                    sc = scps.tile([P, NT], F32, name="sc")
                                    nc.tensor.matmul(
                                        sc,
                                        kt_all[hb:hb + 64, h // 2,
                                               ckg * P:(ckg + 1) * P],
                                        qT_sb[hb:hb + 64, 4 * g + h // 2, :],
                                        start=True, stop=True)
                                    et = att.tile([P, NT], AT_DT, name="et")
                                    nc.scalar.activation(
                                        et, sc,
                                        mybir.ActivationFunctionType.Exp,
                                        bias=mb_sb[:, ckg:ckg + 1], scale=1.0)
                                    nc.tensor.matmul(
                                        op_pair[hh],
                                        v_aug[:, ckg, h, :],
                                        et,
                                        start=(ckg == 0), stop=(ckg == 15))
                        # normalize: recip of sums row, ones-matmul broadcast
                        with nc.allow_low_precision(reason="f32r == f32 bits"):
                            for hh in range(2):
                                rch = rcp.tile([1, NT], MM_DT, name="rc")
                                nc.vector.reciprocal(
                                    rch, op_pair[hh][64:65, :])
                                rcb = rps.tile([64, NT], F32, name="rcb")
                                nc.tensor.matmul(rcb, ones64, rch,
                                                 start=True, stop=True)
                                rcs = rcp.tile([64, NT], F32, name="rcs")
                                nc.vector.tensor_copy(rcs, rcb)
                                nc.vector.tensor_tensor(
                                    out=outp_sb[hp][hh * 64:(hh + 1) * 64, :],
                                    in0=op_pair[hh][0:64, :],
                                    in1=rcs,
                                    op=mybir.AluOpType.mult)

            # ---------- phase 3: out projection + row mask ----------
            with tc.tile_pool(name="outp", bufs=3) as outp, \
                 tc.tile_pool(name="oppr", bufs=3, space="PSUM") as oppr:
                for tc_i in range(4):
                    for Fc in range(2):
                        ps = oppr.tile([P, 512], F32, name="ops2")
                        for dc in range(8):
                            nc.tensor.matmul(
                                ps,
                                outp_sb[dc][:, tc_i * P:(tc_i + 1) * P],
                                wo_sb[:, dc, Fc * 512:(Fc + 1) * 512],
                                start=(dc == 0), stop=(dc == 7))
                        ot = outp.tile([P, 512], F32, name="ot")
                        nc.vector.tensor_scalar_mul(
                            ot, ps, mf_sb[:, tc_i:tc_i + 1])
                        nc.sync.dma_start(
                            out=out[tc_i * P:(tc_i + 1) * P,
                                    Fc * 512:(Fc + 1) * 512],
                            in_=ot)

    _legalize_multiwaits(nc)
    return nc


def _prep_host(x, mask, freqs, Wq, Wk, Wv, Wo, q_norm_w, k_norm_w):
    """Host-side shard + fold. Returns per-core input maps."""
    perm = np.concatenate([np.arange(0, HD, 2), np.arange(1, HD, 2)])  # evens, odds
    swap = np.concatenate([np.arange(32, 64), np.arange(0, 32)])

    def permute_cols(W):  # [D, D] -> per-head column perm
        W4 = W.reshape(D, H, HD)
        return W4[:, :, perm].reshape(D, D)

    W3 = np.concatenate(
        [permute_cols(Wq), permute_cols(Wk), Wv], axis=1)  # [D, 3D]
    W3 = np.ascontiguousarray(W3, dtype=np.float32)
    Wo = np.ascontiguousarray(Wo, dtype=np.float32)

    fp = freqs[:, perm].astype(np.float32)          # [N, 64]
    cos = np.cos(fp)
    sin = np.sin(fp)
    sin_s = sin.copy()
    sin_s[:, :32] = -sin_s[:, :32]                  # rotate-half signs
    wq_p = q_norm_w[perm].astype(np.float32)
    wk_p = k_norm_w[perm].astype(np.float32)
    sc = 1.0 / np.sqrt(HD)
    cos_q = cos * wq_p * sc
    sin_q = sin_s * wq_p[swap] * sc
    cos_k = cos * wk_p
    sin_k = sin_s * wk_p[swap]
    cos_q, sin_q, cos_k, sin_k = (
        np.ascontiguousarray(a, dtype=np.float32)
        for a in (cos_q, sin_q, cos_k, sin_k))

    ident = np.eye(P, dtype=np.float32)

    in_maps = []
    for c in range(8):
        b, s = c // 4, c % 4
        rows = slice(s * NT, (s + 1) * NT)
        xT_c = np.ascontiguousarray(x[b, rows, :].T, dtype=np.float32)
        mbias = np.where(mask[b], 0.0, -1e30).astype(np.float32).reshape(N, 1)
        mf = mask[b, rows].astype(np.float32).reshape(NT, 1)
        in_maps.append(dict(
            xT=xT_c, w3=W3, wo=Wo,
            cosq=np.ascontiguousarray(cos_q[rows]),
            sinq=np.ascontiguousarray(sin_q[rows]),
            cosk=np.ascontiguousarray(cos_k[rows]),
            sink=np.ascontiguousarray(sin_k[rows]),
            mbias=mbias, maskf=mf, ident=ident,
            onesd=np.ones((P, 256), dtype=np.float32),
        ))
    return in_maps


_CACHED = {}


def kernel(x, mask, freqs, Wq, bq, Wk, bk, Wv, bv, Wo, bo,
           q_norm_w, k_norm_w, **_unused):
    x = np.asarray(x, dtype=np.float32)
    mask = np.asarray(mask)
    for name, b_ in (("bq", bq), ("bk", bk), ("bv", bv), ("bo", bo)):
        if np.any(np.asarray(b_) != 0):
            raise NotImplementedError(f"nonzero bias {name} not supported")

    in_maps = _prep_host(np.asarray(x), np.asarray(mask),
                         np.asarray(freqs, dtype=np.float32),
                         np.asarray(Wq, dtype=np.float32),
                         np.asarray(Wk, dtype=np.float32),
                         np.asarray(Wv, dtype=np.float32),
                         np.asarray(Wo, dtype=np.float32),
                         np.asarray(q_norm_w, dtype=np.float32),
                         np.asarray(k_norm_w, dtype=np.float32))

    if "nc" not in _CACHED:
        _CACHED["nc"] = build_kernel()
    res = run_bass_kernel_spmd(_CACHED["nc"], in_maps, core_ids=list(range(8)))

    out = np.empty((B, N, D), dtype=np.float32)
    for c in range(8):
        b, s = c // 4, c % 4
        out[b, s * NT:(s + 1) * NT, :] = res.results[c]["out"]
    return out


if __name__ == "__main__":
    rng = np.random.default_rng(0)
    ins = dict(
        x=rng.standard_normal((B, N, D), dtype=np.float32),
        mask=rng.random((B, N)) < 0.5,
        freqs=rng.standard_normal((N, HD), dtype=np.float32),
        Wq=rng.standard_normal((D, D), dtype=np.float32) * 0.02,
        bq=np.zeros(D, np.float32),
        Wk=rng.standard_normal((D, D), dtype=np.float32) * 0.02,
        bk=np.zeros(D, np.float32),
        Wv=rng.standard_normal((D, D), dtype=np.float32) * 0.02,
        bv=np.zeros(D, np.float32),
        Wo=rng.standard_normal((D, D), dtype=np.float32) * 0.02,
        bo=np.zeros(D, np.float32),
        q_norm_w=np.ones(HD, np.float32),
        k_norm_w=np.ones(HD, np.float32),
    )
    o = kernel(**ins)
    print("out", o.shape, o.dtype, float(np.abs(o).max()))
